# revision 1
# baseline (speedup 1.0000x reference)
"""Trainium2 Bass kernel for nn_AttentionModel (seq2seq LSTM with attention).

Sharding: pure data parallelism over batch (256 -> 8 cores x 32), all
weights replicated (bf16). Per-core layout keeps the hidden/gate dimension
on SBUF partitions and (time, batch) on the free axis so the recurrent
matmuls, elementwise gate math, and attention all use one consistent
layout with no on-device transposes.

Key performance choices:
- gates packed [i, f, o, g] so one Sigmoid covers i,f,o contiguously
  (ACT function switches reload the LUT, ~1.3us each)
- attention scores via a diagonal matmul (c stationary) so the softmax
  lands batch-on-partitions and runs as per-partition-scalar ops
- all copies on DVE; ACT used only for Sigmoid/Tanh/Exp

Self-contained: includes the TileContext wait-split workaround and all
host-side packing. The graded entry point is kernel(**inputs).
"""

import numpy as np
import ml_dtypes

import concourse.bass as bass
import concourse.mybir as mybir
import concourse.tile as tile
from concourse.bass_utils import run_bass_kernel_spmd

BF16 = ml_dtypes.bfloat16
FP8 = ml_dtypes.float8_e4m3
FP32 = mybir.dt.float32
BF = mybir.dt.bfloat16
F8 = mybir.dt.float8e4

# fp8 gate-weight mode: whh_{e,p,d} and wih_d stored fp8 (2x faster
# LDWEIGHTS via FWL); h/atth/inp cast to fp8 for those matmuls.
GATE_FP8 = False

N_CORES = 8
B = 32            # batch per core
T_IN = 10
T_OUT = 25
H = 1024
F = 512
P = 66
G = 4 * H         # 4096 gates
KT = H // 128     # 8  k-tiles over hidden
FT = F // 128     # 4  k-tiles over feature
MT = G // 128     # 32 m-tiles over gates
TCAT = 2 * T_IN + 1   # 21 attention slots
SLOT_DEC = T_IN       # decoder h lives at slot 10

_MAX_WAITS = 1


def _apply_tile_wait_patches():
    """The walrus CoreV3 codegen in this container rejects instructions
    carrying more than one sync-wait command ("Too many sync wait
    commands"). Keep every instruction at <=1 wait by moving excess waits
    onto same-engine nops emitted immediately before the instruction."""
    import bass_rust
    from concourse.vector_clock import ScopedClock

    SyncInfo = bass_rust.SyncInfo

    def _split_waits(nc, inst):
        si = getattr(inst, "sync_info", None)
        if si is None or not si.on_wait or len(si.on_wait) <= _MAX_WAITS:
            return
        if inst.engine == mybir.EngineType.Unassigned:
            return
        waits = list(si.on_wait)
        si.on_wait = waits[:_MAX_WAITS]
        rest = waits[_MAX_WAITS:]
        eng = nc.engines[inst.engine]
        for i in range(0, len(rest), _MAX_WAITS):
            nop = eng.nop(nofuse=True, hint="wait_split")
            nop.ins.sync_info = SyncInfo(
                on_wait=rest[i:i + _MAX_WAITS], on_update=[]
            )

    orig_commit = tile.TileContext._commit_instruction

    def _commit_split(self, inst, lazy_reg_writes=True):
        si = getattr(inst, "sync_info", None)
        if (si is not None and si.on_wait is not None
                and len(si.on_wait) > _MAX_WAITS
                and inst.engine != mybir.EngineType.Unassigned):
            _split_waits(self.nc, inst)
        return orig_commit(self, inst, lazy_reg_writes)

    tile.TileContext._commit_instruction = _commit_split

    def _drain_and_barrier_split(self, tick_clock, wait_clock):
        drain_inst = self.nc.sync.drain()
        wait_clock.add_sem_waits(
            drain_inst.ins, ScopedClock({None: tick_clock.global_clock})
        )
        sync_info = drain_inst.ins.sync_info
        if sync_info is not None and sync_info.on_wait is not None:
            waits = list(sync_info.on_wait)
            if len(waits) > _MAX_WAITS:
                sync_info.on_wait = waits[:_MAX_WAITS]
                rest = waits[_MAX_WAITS:]
                for i in range(0, len(rest), _MAX_WAITS):
                    nop = self.nc.sync.nop(nofuse=True, hint="drain_wait_split")
                    nop.ins.sync_info = SyncInfo(
                        on_wait=rest[i:i + _MAX_WAITS], on_update=[]
                    )
        self.nc.all_engine_barrier()
        assert self.sems is not None
        popped = self.nc._tile_sem_poison_stack.pop()
        assert popped is self._sem_poison
        self.nc.clear_and_free_semaphores(list(self.sems.allocated().values()))
        self.nc.all_engine_barrier()

    tile.TileContext._drain_and_barrier = _drain_and_barrier_split


_apply_tile_wait_patches()

# Opt-in: flip walrus --enable-ldw-opt (hardcoded false in
# bass_utils.bir_verify_and_optimise) by rewriting the command line.
LDW_OPT = False


def _apply_ldw_opt_patch():
    from concourse import bass_utils as _bu
    if getattr(_bu, "_ldw_opt_patched", False):
        return
    _bu._ldw_opt_patched = True
    _orig_run_command = _bu.run_command

    def _run_command_ldw(argv, **kwargs):
        if LDW_OPT:
            argv = ["--enable-ldw-opt=true" if a == "--enable-ldw-opt=false"
                    else a for a in argv]
        return _orig_run_command(argv, **kwargs)

    _bu.run_command = _run_command_ldw


_apply_ldw_opt_patch()


# ------------------------------------------------------------- host packing

# gate reorder: reference packs gates [i, f, g, o]; we use [i, f, o, g]
# so the three sigmoids (i, f, o) are contiguous.
_GPERM = np.concatenate([
    np.arange(0, H),              # i
    np.arange(H, 2 * H),          # f
    np.arange(3 * H, 4 * H),      # o
    np.arange(2 * H, 3 * H),      # g
])


def _pack_T(w, ktiles, mcols, dt=BF16):
    """(mcols, ktiles*128) weight -> transposed tiled layout
    (128, ktiles*mcols) with [p, kt*mcols + m] = w[m, kt*128 + p]."""
    wT = np.ascontiguousarray(w.T).astype(dt)      # (ktiles*128, mcols)
    return np.ascontiguousarray(
        wT.reshape(ktiles, 128, mcols).transpose(1, 0, 2)
        .reshape(128, ktiles * mcols))


def _prep_weights(inputs):
    d = {}
    d["tfT"] = np.ascontiguousarray(inputs["tf_w"].T).astype(BF16)  # (66, 512)
    for nm, wih, whh in (("e", "enc_wih", "enc_whh"),
                         ("p", "encp_wih", "encp_whh"),
                         ("d", "dec_wih", "dec_whh")):
        d[f"wih_{nm}"] = _pack_T(inputs[wih][_GPERM], FT, G)   # (128, 4*4096)
        d[f"whh_{nm}"] = _pack_T(inputs[whh][_GPERM], KT, G)   # (128, 8*4096)
    d["linT"] = _pack_T(inputs["lin_w"], KT, F)          # (128, 8*512)
    d["tpT"] = _pack_T(inputs["tp_w"], KT, P)            # (128, 8*66)
    d["b_tf"] = inputs["tf_b"].astype(np.float32)
    for nm, bi, bh in (("e", "enc_bih", "enc_bhh"),
                       ("p", "encp_bih", "encp_bhh"),
                       ("d", "dec_bih", "dec_bhh")):
        d[f"b_{nm}"] = (inputs[bi] + inputs[bh]).astype(np.float32)[_GPERM]
    d["b_lin"] = inputs["lin_b"].astype(np.float32)
    d["b_tp"] = inputs["tp_b"].astype(np.float32)
    return d


def _bias_flags(w):
    return tuple(bool(np.any(w[k])) for k in
                 ("b_tf", "b_e", "b_p", "b_d", "b_lin", "b_tp"))


# ------------------------------------------------------------ device build

def build_model(bias_flags=(False,) * 6, loop_iters=1, gate_fp8=GATE_FP8,
                ablate=(), warm_fillers=True):
    has_btf, has_be, has_bp, has_bd, has_blin, has_btp = bias_flags
    any_bias = any(bias_flags)

    nc = bass.Bass()

    xT_d = nc.dram_tensor("xT", [P, T_IN * B], BF, kind="ExternalInput")
    zT_d = nc.dram_tensor("zT", [P, T_IN * B], BF, kind="ExternalInput")
    residT_d = nc.dram_tensor("residT", [P, T_OUT * B], FP32,
                              kind="ExternalInput")
    tfT_d = nc.dram_tensor("tfT", [P, F], BF, kind="ExternalInput")
    GDT = F8 if gate_fp8 else BF
    wih_d_d = {}
    whh_d_d = {}
    for nm in ("e", "p", "d"):
        wih_d_d[nm] = nc.dram_tensor(f"wih_{nm}", [128, FT * G],
                                     GDT if nm == "d" else BF,
                                     kind="ExternalInput")
        whh_d_d[nm] = nc.dram_tensor(f"whh_{nm}", [128, KT * G], GDT,
                                     kind="ExternalInput")
    linT_d = nc.dram_tensor("linT", [128, KT * F], BF, kind="ExternalInput")
    dmask_d = nc.dram_tensor("dmask", [B, B * TCAT], BF, kind="ExternalInput")
    tpT_d = nc.dram_tensor("tpT", [128, KT * P], BF, kind="ExternalInput")
    bias_d = {}
    if has_btf:
        bias_d["b_tf"] = nc.dram_tensor("b_tf", [1, F], BF,
                                        kind="ExternalInput")
    if has_be:
        bias_d["b_e"] = nc.dram_tensor("b_e", [1, G], BF,
                                       kind="ExternalInput")
    if has_bp:
        bias_d["b_p"] = nc.dram_tensor("b_p", [1, G], BF,
                                       kind="ExternalInput")
    if has_bd:
        bias_d["b_d"] = nc.dram_tensor("b_d", [1, G], BF,
                                       kind="ExternalInput")
    if has_blin:
        bias_d["b_lin"] = nc.dram_tensor("b_lin", [1, F], BF,
                                         kind="ExternalInput")
    if has_btp:
        bias_d["b_tp"] = nc.dram_tensor("b_tp", [1, P], BF,
                                        kind="ExternalInput")
    out_d = nc.dram_tensor("oT", [P, T_OUT * B], FP32, kind="ExternalOutput")

    with tile.TileContext(nc) as tc:
        with (
            tc.tile_pool(name="singles", bufs=1) as singles,
            tc.tile_pool(name="wih_pool", bufs=1) as wih_pool,
            tc.tile_pool(name="whh_pool", bufs=1) as whh_pool,
            tc.tile_pool(name="gx_pool", bufs=2) as gx_pool,
            tc.tile_pool(name="ew_pool", bufs=4) as ew_pool,
            tc.tile_pool(name="pA_pool", bufs=1, space="PSUM") as pA_pool,
            tc.tile_pool(name="pB_pool", bufs=1, space="PSUM") as pB_pool,
            tc.tile_pool(name="pC_pool", bufs=1, space="PSUM") as pC_pool,
            tc.tile_pool(name="mps_pool", bufs=2, space="PSUM") as mps_pool,
        ):
            def body(_it=None):
                # ------------- constant/static loads --------------------
                tfT = singles.tile([P, F], BF, tag="tfT")
                nc.sync.dma_start(out=tfT, in_=tfT_d[:, :])
                xT = singles.tile([P, T_IN * B], BF, tag="xT")
                nc.sync.dma_start(out=xT, in_=xT_d[:, :])
                zT = singles.tile([P, T_IN * B], BF, tag="zT")
                nc.sync.dma_start(out=zT, in_=zT_d[:, :])
                residT = singles.tile([P, T_OUT * B], FP32, tag="residT")
                nc.sync.dma_start(out=residT, in_=residT_d[:, :])
                linT = singles.tile([128, KT * F], BF, tag="linT")
                for kt in range(KT):
                    nc.sync.dma_start(out=linT[:, kt * F:(kt + 1) * F],
                                      in_=linT_d[:, kt * F:(kt + 1) * F])
                tpT = singles.tile([128, KT * P], BF, tag="tpT")
                nc.sync.dma_start(out=tpT, in_=tpT_d[:, :])
                dmask = singles.tile([B, B * TCAT], BF, tag="dmask")
                nc.sync.dma_start(out=dmask, in_=dmask_d[:, :])

                bias_sb = {}
                for key, dram in bias_d.items():
                    t = singles.tile(list(dram.shape), BF, tag=key)
                    nc.sync.dma_start(out=t, in_=dram[:, :])
                    bias_sb[key] = t

                ones32 = singles.tile([B, 128], BF, tag="ones32")
                nc.vector.memset(ones32, 1.0)
                ident = singles.tile([128, 128], BF, tag="ident")
                from concourse.masks import make_identity
                make_identity(nc, ident)
                if any_bias:
                    ones_n = singles.tile([1, T_IN * B], BF, tag="ones_n")
                    nc.vector.memset(ones_n, 1.0)

                cat = singles.tile([128, KT, B, TCAT], BF, tag="cat")
                c_e = singles.tile([128, KT, B, 1], FP32, tag="c_e")
                c_p = singles.tile([128, KT, B, 1], FP32, tag="c_p")

                c_bf = singles.tile([128, KT, B, 1], BF, tag="c_bf")
                atth = singles.tile([128, KT, B], GDT, tag="atth")
                inp_bf = singles.tile([128, FT, B], GDT, tag="inp_bf")
                e_bc = singles.tile([128, B * TCAT], BF, tag="e_bc")
                masked32 = singles.tile([B, B * TCAT], FP32, tag="masked32")
                scoresbt = singles.tile([B, TCAT], FP32, tag="scoresbt")
                neg_mx = singles.tile([B, 1], FP32, tag="neg_mx")
                e32 = singles.tile([B, TCAT], FP32, tag="e32")
                ssum32 = singles.tile([B, 1], FP32, tag="ssum32")
                rs32 = singles.tile([B, 1], FP32, tag="rs32")
                attw32 = singles.tile([B, TCAT], BF, tag="attw32")
                aw_m = singles.tile([B, B * TCAT], BF, tag="aw_m")
                gw_sb = singles.tile([128, MT, B], BF, tag="gw_sb")
                gA_sb = singles.tile([128, MT, B], BF, tag="gA_sb")
                prod2 = singles.tile([128, B * TCAT], BF, tag="prod2")
                oT_sb = singles.tile([P, T_OUT * B], FP32, tag="oT_sb")
                h8 = None
                if gate_fp8:
                    h8 = singles.tile([128, KT, B], F8, tag="h8")

                xf = singles.tile([128, FT, T_IN * B], BF, tag="xf")
                zf = singles.tile([128, FT, T_IN * B], BF, tag="zf")

                # ------------- ToFeature --------------------------------
                def to_feature(src, dst):
                    for ft in range(FT):
                        ps = mps_pool.tile([128, T_IN * B], FP32, tag="mps")
                        nc.tensor.matmul(ps, tfT[:, ft * 128:(ft + 1) * 128],
                                         src[:, :], start=True,
                                         stop=not has_btf)
                        if has_btf:
                            nc.tensor.matmul(
                                ps,
                                bias_sb["b_tf"][0:1, ft * 128:(ft + 1) * 128],
                                ones_n[0:1, :], start=False, stop=True)
                        nc.vector.tensor_copy(out=dst[:, ft, :], in_=ps)

                to_feature(xT, xf)
                to_feature(zT, zf)

                # ------------- encoder gates_x precompute ----------------
                # gx layout: (128, T_IN, MT, B) so per-step slices are
                # contiguous.
                def gates_x(wih_sb, src, dst, bkey):
                    for mt in range(MT):
                        ps = mps_pool.tile([128, T_IN * B], FP32, tag="mps")
                        for kt in range(FT):
                            nc.tensor.matmul(
                                ps,
                                wih_sb[:, kt * G + mt * 128:
                                       kt * G + (mt + 1) * 128],
                                src[:, kt, :],
                                start=(kt == 0),
                                stop=(kt == FT - 1 and bkey is None),
                            )
                        if bkey is not None:
                            nc.tensor.matmul(
                                ps, bias_sb[bkey][0:1, mt * 128:(mt + 1) * 128],
                                ones_n[0:1, :], start=False, stop=True)
                        nc.vector.tensor_copy(
                            out=dst[:, :, mt, :],
                            in_=ps.rearrange("p (t b) -> p t b", b=B))

                wih_e = wih_pool.tile([128, FT * G], BF, tag="wih")
                for kt in range(FT):
                    nc.sync.dma_start(out=wih_e[:, kt * G:(kt + 1) * G],
                                      in_=wih_d_d["e"][:, kt * G:(kt + 1) * G])
                gx_e = gx_pool.tile([128, T_IN, MT, B], BF, tag="gx")
                gates_x(wih_e, xf, gx_e, "b_e" if has_be else None)

                wih_p = wih_pool.tile([128, FT * G], BF, tag="wih")
                for kt in range(FT):
                    nc.sync.dma_start(out=wih_p[:, kt * G:(kt + 1) * G],
                                      in_=wih_d_d["p"][:, kt * G:(kt + 1) * G])
                gx_p = gx_pool.tile([128, T_IN, MT, B], BF, tag="gx")
                gates_x(wih_p, zf, gx_p, "b_p" if has_bp else None)

                wih_dd = wih_pool.tile([128, FT * G], GDT, tag="wih")
                for kt in range(FT):
                    nc.sync.dma_start(out=wih_dd[:, kt * G:(kt + 1) * G],
                                      in_=wih_d_d["d"][:, kt * G:(kt + 1) * G])


                def pe_filler(dep_ap):
                    # Tiny matmul dependent on a just-produced DVE/ACT tile.
                    # Keeps the PE HAM activity window busy during long
                    # DVE/ACT chains so the next real matmul block starts at
                    # 2.4 GHz instead of re-warming from 1.2 GHz.
                    if not warm_fillers:
                        return
                    fps = mps_pool.tile([1, 8], FP32, tag="mps")
                    nc.tensor.matmul(fps[:, 0:1], dep_ap, dep_ap,
                                     start=True, stop=True)

                # ------------- LSTM gate elementwise ---------------------
                # gates packed [i, f, o, g]: one Sigmoid over [0:3H], one
                # Tanh over g; ACT never does copies.
                def lstm_tail(gsrc, c_tile, h_out, first_step,
                              emit_cbf=False):
                    sio = ew_pool.tile([128, 3 * KT, B], FP32, tag="ew")
                    tg = ew_pool.tile([128, KT, B], FP32, tag="ew")
                    nc.scalar.activation(
                        out=sio, in_=gsrc[:, 0:3 * KT, :],
                        func=mybir.ActivationFunctionType.Sigmoid)
                    nc.scalar.activation(
                        out=tg, in_=gsrc[:, 3 * KT:4 * KT, :],
                        func=mybir.ActivationFunctionType.Tanh)
                    si = sio[:, 0:KT, :]
                    sf = sio[:, KT:2 * KT, :]
                    so = sio[:, 2 * KT:3 * KT, :]
                    cs = c_tile[:, :, :, 0]
                    pe_filler(sio[:, 0:1, 0:1])
                    # tg <- i*g
                    nc.vector.tensor_mul(tg, si, tg)
                    if first_step:
                        nc.vector.tensor_copy(out=cs, in_=tg)
                    else:
                        nc.vector.tensor_mul(cs, sf, cs)
                        nc.vector.tensor_add(cs, cs, tg)
                    th = ew_pool.tile([128, KT, B], FP32, tag="ew")
                    nc.scalar.activation(
                        out=th, in_=cs,
                        func=mybir.ActivationFunctionType.Tanh)
                    if emit_cbf:
                        # next decoder step's attention input, cast while
                        # ACT computes tanh / h
                        nc.vector.tensor_copy(out=c_bf, in_=c_tile)
                    nc.vector.tensor_mul(h_out, so, th)

                # ------------- encoder chains ----------------------------
                def run_encoder(whh_sb, gx, c_tile, slot0):
                    for t in range(T_IN):
                        slot = slot0 + t
                        h_out = cat[:, :, :, slot]
                        if t == 0:
                            lstm_tail(gx[:, 0, :, :], c_tile, h_out, True)
                            if gate_fp8:
                                nc.vector.tensor_copy(out=h8, in_=h_out)
                            continue
                        gps = pC_pool.tile([128, MT, B], FP32, tag="pC")
                        prev = h8 if gate_fp8 else cat[:, :, :, slot - 1]
                        for mt in range(MT):
                            for kt in range(KT):
                                nc.tensor.matmul(
                                    gps[:, mt, :],
                                    whh_sb[:, kt * G + mt * 128:
                                           kt * G + (mt + 1) * 128],
                                    prev[:, kt, :],
                                    start=(kt == 0), stop=False,
                                )
                            # + gx[t] via identity matmul: the PE does the
                            # add inside the accumulation, keeping the DVE
                            # add off the serial elementwise tail
                            nc.tensor.matmul(
                                gps[:, mt, :], ident,
                                gx[:, t, mt, :],
                                start=False, stop=True)
                        lstm_tail(gps, c_tile, h_out, False)
                        if gate_fp8:
                            nc.vector.tensor_copy(out=h8, in_=h_out)

                whh_e = whh_pool.tile([128, KT * G], GDT, tag="whh")
                for kt in range(KT):
                    nc.sync.dma_start(out=whh_e[:, kt * G:(kt + 1) * G],
                                      in_=whh_d_d["e"][:, kt * G:(kt + 1) * G])
                if "enc" not in ablate:
                    run_encoder(whh_e, gx_e, c_e, 0)
                else:
                    nc.vector.memset(cat, 0.01)
                    nc.vector.memset(c_e, 0.01)
                    nc.vector.memset(c_p, 0.01)

                whh_p = whh_pool.tile([128, KT * G], GDT, tag="whh")
                for kt in range(KT):
                    nc.sync.dma_start(out=whh_p[:, kt * G:(kt + 1) * G],
                                      in_=whh_d_d["p"][:, kt * G:(kt + 1) * G])
                if "enc" not in ablate:
                    run_encoder(whh_p, gx_p, c_p, T_IN + 1)

                # ------------- decoder ----------------------------------
                whh_dd = whh_pool.tile([128, KT * G], GDT, tag="whh")
                for kt in range(KT):
                    nc.sync.dma_start(out=whh_dd[:, kt * G:(kt + 1) * G],
                                      in_=whh_d_d["d"][:, kt * G:(kt + 1) * G])

                dec_hs = gx_pool.tile([128, KT, T_OUT, B], BF, tag="gx")

                nc.vector.tensor_copy(out=cat[:, :, :, SLOT_DEC],
                                      in_=cat[:, :, :, T_IN - 1])
                c_d = c_e
                nc.vector.tensor_copy(out=c_bf, in_=c_d)

                nhalf = B // 2

                dec_steps = 0 if "dec" in ablate else T_OUT
                for t in range(dec_steps):
                    # lin: inp = h @ lin_w.T
                    ips = mps_pool.tile([128, FT, B], FP32, tag="mps")
                    for mt in range(FT):
                        for kt in range(KT):
                            nc.tensor.matmul(
                                ips[:, mt, :],
                                linT[:, kt * F + mt * 128:
                                     kt * F + (mt + 1) * 128],
                                cat[:, kt, :, SLOT_DEC],
                                start=(kt == 0),
                                stop=(kt == KT - 1 and not has_blin),
                            )
                        if has_blin:
                            nc.tensor.matmul(
                                ips[:, mt, :],
                                bias_sb["b_lin"][0:1, mt * 128:(mt + 1) * 128],
                                ones_n[0:1, 0:B], start=False, stop=True)
                    nc.vector.tensor_copy(out=inp_bf, in_=ips)

                    def emit_gpw():
                        # wih gates into their own PSUM region; emitted after
                        # the scores matmuls so PE stays busy while DVE/ACT
                        # run the softmax chain. Merged by one add later.
                        gpw = pB_pool.tile([128, MT, B], FP32, tag="pB")
                        for mt in range(MT):
                            for kt in range(FT):
                                nc.tensor.matmul(
                                    gpw[:, mt, :],
                                    wih_dd[:, kt * G + mt * 128:
                                           kt * G + (mt + 1) * 128],
                                    inp_bf[:, kt, :],
                                    start=(kt == 0),
                                    stop=(kt == FT - 1 and not has_bd))
                            if has_bd:
                                nc.tensor.matmul(
                                    gpw[:, mt, :],
                                    bias_sb["b_d"][0:1,
                                                   mt * 128:(mt + 1) * 128],
                                    ones_n[0:1, 0:B], start=False, stop=True)
                        nc.vector.tensor_copy(out=gw_sb, in_=gpw)

                    if t == 0 and "attn" in ablate:
                        nc.vector.memset(atth, 0.01)
                    if "attn" in ablate:
                        emit_gpw()
                    if "attn" not in ablate:
                        # scores via diagonal matmul: out[b,(b',t)] =
                        # sum_h c[h,b] cat[h,b',t]; diagonal extracted by
                        # mask-multiply + strided reduce. Scores land
                        # batch-on-partitions -> softmax is 4 cheap ops.
                        scd = pA_pool.tile([B, 2, 512], FP32, tag="pA")
                        for kt in range(KT):
                            nc.tensor.matmul(
                                scd[:, 0, 0:nhalf * TCAT],
                                c_bf[:, kt, :, 0], cat[:, kt, 0:nhalf, :],
                                start=(kt == 0), stop=(kt == KT - 1))
                            nc.tensor.matmul(
                                scd[:, 1, 0:nhalf * TCAT],
                                c_bf[:, kt, :, 0], cat[:, kt, nhalf:B, :],
                                start=(kt == 0), stop=(kt == KT - 1))
                        emit_gpw()
                        nc.vector.tensor_mul(
                            masked32.rearrange("p (c n) -> p c n", c=2),
                            scd[:, :, 0:nhalf * TCAT],
                            dmask.rearrange("p (c n) -> p c n", c=2))
                        nc.vector.tensor_reduce(
                            scoresbt,
                            masked32.rearrange("p (b t) -> p t b", t=TCAT),
                            axis=mybir.AxisListType.X, op=mybir.AluOpType.add)
                        pe_filler(scoresbt[:, 0:1])
                        nc.vector.tensor_reduce(
                            neg_mx, scoresbt, axis=mybir.AxisListType.X,
                            op=mybir.AluOpType.max, negate=True)
                        nc.scalar.activation(
                            out=e32, in_=scoresbt,
                            func=mybir.ActivationFunctionType.Exp,
                            bias=neg_mx, accum_out=ssum32)
                        pe_filler(e32[:, 0:1])
                        nc.vector.reciprocal(rs32, ssum32)
                        nc.vector.tensor_scalar_mul(attw32, e32, rs32)
                        # broadcast attw to all partitions without a DMA:
                        # aw_m[b, (b', t)] = attw[b, t] * dmask, then a K=32
                        # ones matmul sums over b leaving attw[b', t]
                        # replicated on every partition.
                        nc.vector.tensor_mul(
                            aw_m, dmask,
                            attw32.unsqueeze(1).to_broadcast((B, B, TCAT)))
                        eall = pA_pool.tile([128, 2, 512], FP32, tag="pA")
                        nc.tensor.matmul(eall[:, 0, 0:nhalf * TCAT],
                                         ones32, aw_m[:, 0:nhalf * TCAT],
                                         start=True, stop=True)
                        nc.tensor.matmul(eall[:, 1, 0:nhalf * TCAT],
                                         ones32, aw_m[:, nhalf * TCAT:],
                                         start=True, stop=True)
                        nc.vector.tensor_copy(out=e_bc[:, 0:nhalf * TCAT],
                                              in_=eall[:, 0, 0:nhalf * TCAT])
                        nc.vector.tensor_copy(out=e_bc[:, nhalf * TCAT:],
                                              in_=eall[:, 1, 0:nhalf * TCAT])

                        # context: atth[:, kt, :] = sum_t cat[:, kt] * attw
                        ctx = ew_pool.tile([128, KT, B], FP32, tag="ew")
                        for kt in range(KT):
                            nc.vector.tensor_mul(
                                prod2, cat[:, kt, :, :],
                                e_bc.rearrange("p (b t) -> p b t", t=TCAT))
                            nc.vector.tensor_reduce(
                                ctx[:, kt, :],
                                prod2.rearrange("p (b t) -> p b t", t=TCAT),
                                axis=mybir.AxisListType.X,
                                op=mybir.AluOpType.add)
                            nc.vector.tensor_copy(out=atth[:, kt, :],
                                                  in_=ctx[:, kt, :])

                    # whh part of gates, then merge the wih psum in
                    # half-A: whh kt0-3 into the freed wih psum slot;
                    # starts while the kt4-7 context slices still reduce
                    gpsA = pB_pool.tile([128, MT, B], FP32, tag="pB")
                    for mt in range(MT):
                        for kt in range(4):
                            nc.tensor.matmul(
                                gpsA[:, mt, :],
                                whh_dd[:, kt * G + mt * 128:
                                       kt * G + (mt + 1) * 128],
                                atth[:, kt, :],
                                start=(kt == 0), stop=(kt == 3))
                    nc.vector.tensor_copy(out=gA_sb, in_=gpsA)
                    # half-B: ident folds of wih + half-A, then whh kt4-7
                    gps = pC_pool.tile([128, MT, B], FP32, tag="pC")
                    for mt in range(MT):
                        nc.tensor.matmul(
                            gps[:, mt, :], ident,
                            gw_sb[:, mt, :],
                            start=True, stop=False)
                        nc.tensor.matmul(
                            gps[:, mt, :], ident,
                            gA_sb[:, mt, :],
                            start=False, stop=False)
                        for kt in range(4, KT):
                            nc.tensor.matmul(
                                gps[:, mt, :],
                                whh_dd[:, kt * G + mt * 128:
                                       kt * G + (mt + 1) * 128],
                                atth[:, kt, :],
                                start=False, stop=(kt == KT - 1))

                    lstm_tail(gps, c_d, cat[:, :, :, SLOT_DEC], False,
                              emit_cbf=(t < T_OUT - 1))
                    nc.vector.tensor_copy(out=dec_hs[:, :, t, :],
                                          in_=cat[:, :, :, SLOT_DEC])

                # ------------- ToPose + residual ------------------------
                if "dec" in ablate:
                    return
                ops = pA_pool.tile([P, 2, 512], FP32, tag="pA")
                chunks = [(0, 13), (13, 12)]
                for ci, (t0, tn) in enumerate(chunks):
                    n = tn * B
                    for kt in range(KT):
                        nc.tensor.matmul(
                            ops[:, ci, 0:n],
                            tpT[:, kt * P:(kt + 1) * P],
                            dec_hs[:, kt, t0:t0 + tn, :].rearrange(
                                "p t b -> p (t b)"),
                            start=(kt == 0),
                            stop=(kt == KT - 1 and not has_btp))
                    if has_btp:
                        nc.tensor.matmul(
                            ops[:, ci, 0:n], bias_sb["b_tp"][0:1, :],
                            ones_n[0:1, 0:n], start=False, stop=True)
                    nc.vector.tensor_add(
                        oT_sb[:, t0 * B:t0 * B + n],
                        ops[:, ci, 0:n],
                        residT[:, t0 * B:t0 * B + n])
                nc.sync.dma_start(out=out_d[:, :], in_=oT_sb)

            if loop_iters > 1:
                with tc.For_i(0, loop_iters, 1, name="rep"):
                    body()
            else:
                body()

    return nc


# ------------------------------------------------------------- entry point

_model_cache = {}


def _get_model(key):
    if key not in _model_cache:
        bias_flags, gate_fp8 = key
        _model_cache[key] = build_model(bias_flags, gate_fp8=gate_fp8)
    return _model_cache[key]


def make_in_maps(inputs, gate_fp8=GATE_FP8):
    """Host-side packing: returns per-core input maps."""
    w = _prep_weights(inputs)
    if gate_fp8:
        for nm in ("e", "p", "d"):
            w[f"whh_{nm}"] = w[f"whh_{nm}"].astype(FP8)
        w["wih_d"] = w["wih_d"].astype(FP8)
    flags = _bias_flags(w)
    x = np.asarray(inputs["x"], dtype=np.float32)
    z = np.asarray(inputs["z"], dtype=np.float32)
    fr = np.asarray(inputs["for_resid"], dtype=np.float32)

    dmask = np.zeros((B, B, TCAT), dtype=np.float32)
    for b in range(B):
        dmask[b, b, :] = 1.0
    shared = {
        "tfT": w["tfT"], "linT": w["linT"], "tpT": w["tpT"],
        "dmask": np.ascontiguousarray(
            dmask.reshape(B, B * TCAT)).astype(BF16),
    }
    for nm in ("e", "p", "d"):
        shared[f"wih_{nm}"] = w[f"wih_{nm}"]
        shared[f"whh_{nm}"] = w[f"whh_{nm}"]
    names = ("b_tf", "b_e", "b_p", "b_d", "b_lin", "b_tp")
    for f, name in zip(flags, names):
        if f:
            shared[name] = np.ascontiguousarray(
                w[name][None, :]).astype(BF16)

    in_maps = []
    for c in range(N_CORES):
        sl = slice(c * B, (c + 1) * B)
        m = dict(shared)
        m["xT"] = np.ascontiguousarray(
            x[sl].transpose(2, 1, 0).reshape(P, T_IN * B)).astype(BF16)
        m["zT"] = np.ascontiguousarray(
            z[sl].transpose(2, 1, 0).reshape(P, T_IN * B)).astype(BF16)
        m["residT"] = np.ascontiguousarray(
            fr[sl].transpose(2, 1, 0).reshape(P, T_OUT * B))
        in_maps.append(m)
    return in_maps, flags


def unshard_output(results):
    outs = []
    for c in range(N_CORES):
        oT = np.asarray(results[c]["oT"])  # (66, 800)
        outs.append(oT.reshape(P, T_OUT, B).transpose(2, 1, 0))
    return np.ascontiguousarray(np.concatenate(outs, axis=0),
                                dtype=np.float32)


def kernel(**inputs) -> np.ndarray:
    in_maps, flags = make_in_maps(inputs)
    nc = _get_model((flags, GATE_FP8))
    res = run_bass_kernel_spmd(nc, in_maps, core_ids=list(range(N_CORES)))
    return unshard_output(res.results)



# revision 24
# speedup vs baseline: 1.3710x; 1.3710x over previous
"""Trainium2 Bass kernel for nn_AttentionModel (seq2seq LSTM with attention).

Sharding: pure data parallelism over batch (256 -> 8 cores x 32), all
weights replicated. Per-core layout keeps the hidden/gate dimension on
SBUF partitions and (time, batch) on the free axis.

Key performance choices (v2):
- all gate weights (wih/whh/lin) in fp8 e3m4, host-scaled into the e3m4
  normal range; moving operands (h/atth/inp/xf) cast to fp8 with their own
  power-of-2 scales. Fast-weight-load makes the N=32 recurrent matmuls
  ~2x faster, which is the PE bottleneck.
- ACT table-set discipline: encoders use {Sigmoid, Tanh} (one set); the
  decoder uses only {Tanh, Exp, Copy} (all in exp_and_others), with
  sigmoid computed as (tanh(x/2)+1)/2 via fused scalar_tensor_tensor ops.
  This removes 2 x ~2.7us of ACT table reloads per decoder step.
- decoder wih and whh matmuls accumulate into ONE psum group per m-tile
  (no identity-merge matmuls, no gate-psum spill copies).
- encoder x-gates are added on DVE (one tensor_tensor add per step)
  instead of identity matmuls; the two encoder chains are interleaved so
  each chain's elementwise tail hides under the other's matmuls.
- attention: scores via diagonal matmul; softmax lands batch-on-partition;
  the context reduce writes the fp8 whh operand directly, pipelined
  k-tile by k-tile against the whh matmuls.

Self-contained: includes the TileContext wait-split workaround and all
host-side packing. The graded entry point is kernel(**inputs).
"""

import numpy as np
import ml_dtypes

import concourse.bass as bass
import concourse.mybir as mybir
import concourse.tile as tile
from concourse.bass_utils import run_bass_kernel_spmd

BF16 = ml_dtypes.bfloat16
F8E3 = ml_dtypes.float8_e3m4
FP32 = mybir.dt.float32
BF = mybir.dt.bfloat16
F8 = mybir.dt.float8e3

import os
FP8 = os.environ.get("K_FP8", "1") == "1"   # fp8 e3m4 gate weights + movers
DEBUG_TAPS = os.environ.get("K_DEBUG_TAPS", "0") == "1"

N_CORES = 8
B = 32            # batch per core
T_IN = 10
T_OUT = 25
H = 1024
F = 512
P = 66
G = 4 * H         # 4096 gates
KT = H // 128     # 8  k-tiles over hidden
FT = F // 128     # 4  k-tiles over feature
MT = G // 128     # 32 m-tiles over gates
TCAT = 2 * T_IN + 1   # 21 attention slots
SLOT_DEC = T_IN       # decoder h lives at slot 10

# fp8 scaling: weights *SW (wih *SWI), moving operands *SH / *SX so every
# gate psum carries the same SG = 256 factor, undone in the ACT scale.
SW = 64.0         # whh / lin weight scale
SWI = 128.0       # wih weight scale
SH = 4.0          # h / atth scale
SX = 2.0          # xf / inp scale
SG = 256.0        # gate psum scale ( = SW*SH = SWI*SX )
E3MAX = 15.5

_MAX_WAITS = 1

ACT = mybir.ActivationFunctionType
ALU = mybir.AluOpType
AXX = mybir.AxisListType


def _apply_tile_wait_patches():
    """The walrus CoreV3 codegen in this container rejects instructions
    carrying more than one sync-wait command ("Too many sync wait
    commands"). Keep every instruction at <=1 wait by moving excess waits
    onto same-engine nops emitted immediately before the instruction."""
    import bass_rust
    from concourse.vector_clock import ScopedClock

    SyncInfo = bass_rust.SyncInfo

    def _split_waits(nc, inst):
        si = getattr(inst, "sync_info", None)
        if si is None or not si.on_wait or len(si.on_wait) <= _MAX_WAITS:
            return
        if inst.engine == mybir.EngineType.Unassigned:
            return
        waits = list(si.on_wait)
        si.on_wait = waits[:_MAX_WAITS]
        rest = waits[_MAX_WAITS:]
        eng = nc.engines[inst.engine]
        for i in range(0, len(rest), _MAX_WAITS):
            nop = eng.nop(nofuse=True, hint="wait_split")
            nop.ins.sync_info = SyncInfo(
                on_wait=rest[i:i + _MAX_WAITS], on_update=[]
            )

    orig_commit = tile.TileContext._commit_instruction

    def _commit_split(self, inst, lazy_reg_writes=True):
        si = getattr(inst, "sync_info", None)
        if (si is not None and si.on_wait is not None
                and len(si.on_wait) > _MAX_WAITS
                and inst.engine != mybir.EngineType.Unassigned):
            _split_waits(self.nc, inst)
        return orig_commit(self, inst, lazy_reg_writes)

    tile.TileContext._commit_instruction = _commit_split

    def _drain_and_barrier_split(self, tick_clock, wait_clock):
        drain_inst = self.nc.sync.drain()
        wait_clock.add_sem_waits(
            drain_inst.ins, ScopedClock({None: tick_clock.global_clock})
        )
        sync_info = drain_inst.ins.sync_info
        if sync_info is not None and sync_info.on_wait is not None:
            waits = list(sync_info.on_wait)
            if len(waits) > _MAX_WAITS:
                sync_info.on_wait = waits[:_MAX_WAITS]
                rest = waits[_MAX_WAITS:]
                for i in range(0, len(rest), _MAX_WAITS):
                    nop = self.nc.sync.nop(nofuse=True, hint="drain_wait_split")
                    nop.ins.sync_info = SyncInfo(
                        on_wait=rest[i:i + _MAX_WAITS], on_update=[]
                    )
        self.nc.all_engine_barrier()
        assert self.sems is not None
        popped = self.nc._tile_sem_poison_stack.pop()
        assert popped is self._sem_poison
        self.nc.clear_and_free_semaphores(list(self.sems.allocated().values()))
        self.nc.all_engine_barrier()

    tile.TileContext._drain_and_barrier = _drain_and_barrier_split


_apply_tile_wait_patches()


# ------------------------------------------------------------- host packing

# gate reorder: reference packs gates [i, f, g, o]; we use [i, f, o, g]
# so one activation covers i,f,o contiguously.
_GPERM = np.concatenate([
    np.arange(0, H),              # i
    np.arange(H, 2 * H),          # f
    np.arange(3 * H, 4 * H),      # o
    np.arange(2 * H, 3 * H),      # g
])


def _pack_T(w, ktiles, mcols, dt=BF16, scale=1.0):
    """(mcols, ktiles*128) weight -> transposed tiled layout
    (128, ktiles*mcols) with [p, kt*mcols + m] = w[m, kt*128 + p]."""
    w = np.asarray(w, np.float32) * scale
    if dt is F8E3:
        w = np.clip(w, -E3MAX, E3MAX)
    wT = np.ascontiguousarray(w.T).astype(dt)      # (ktiles*128, mcols)
    return np.ascontiguousarray(
        wT.reshape(ktiles, 128, mcols).transpose(1, 0, 2)
        .reshape(128, ktiles * mcols))


def _prep_weights(inputs):
    wdt = F8E3 if FP8 else BF16
    sw = SW if FP8 else 1.0
    swi = SWI if FP8 else 1.0
    d = {}
    d["tfT"] = np.ascontiguousarray(inputs["tf_w"].T).astype(BF16)  # (66, 512)
    for nm, wih, whh in (("e", "enc_wih", "enc_whh"),
                         ("p", "encp_wih", "encp_whh"),
                         ("d", "dec_wih", "dec_whh")):
        d[f"wih_{nm}"] = _pack_T(inputs[wih][_GPERM], FT, G, wdt, swi)
        d[f"whh_{nm}"] = _pack_T(inputs[whh][_GPERM], KT, G, wdt, sw)
    d["linT"] = _pack_T(inputs["lin_w"], KT, F, wdt, sw)
    d["tpT"] = _pack_T(inputs["tp_w"], KT, P)            # (128, 8*66)
    d["b_tf"] = inputs["tf_b"].astype(np.float32)
    sg = SG if FP8 else 1.0
    for nm, bi, bh in (("e", "enc_bih", "enc_bhh"),
                       ("p", "encp_bih", "encp_bhh"),
                       ("d", "dec_bih", "dec_bhh")):
        d[f"b_{nm}"] = ((inputs[bi] + inputs[bh]).astype(np.float32)
                        [_GPERM] * sg)
    d["b_lin"] = inputs["lin_b"].astype(np.float32) * sg
    d["b_tp"] = inputs["tp_b"].astype(np.float32)
    return d


def _bias_flags(w):
    return tuple(bool(np.any(w[k])) for k in
                 ("b_tf", "b_e", "b_p", "b_d", "b_lin", "b_tp"))


# ------------------------------------------------------------ device build

def build_model(bias_flags=(False,) * 6, loop_iters=1, ablate=(),
                warm_fillers=True):
    has_btf, has_be, has_bp, has_bd, has_blin, has_btp = bias_flags
    any_bias = any(bias_flags)
    WDT = F8 if FP8 else BF
    MDT = F8 if FP8 else BF   # moving operand dtype for gate matmuls
    sg_inv = 1.0 / (SG if FP8 else 1.0)
    sh = SH if FP8 else 1.0
    sx = SX if FP8 else 1.0

    nc = bass.Bass()

    xT_d = nc.dram_tensor("xT", [P, T_IN * B], BF, kind="ExternalInput")
    zT_d = nc.dram_tensor("zT", [P, T_IN * B], BF, kind="ExternalInput")
    residT_d = nc.dram_tensor("residT", [P, T_OUT * B], FP32,
                              kind="ExternalInput")
    tfT_d = nc.dram_tensor("tfT", [P, F], BF, kind="ExternalInput")
    wih_d_d = {}
    whh_d_d = {}
    for nm in ("e", "p", "d"):
        wih_d_d[nm] = nc.dram_tensor(f"wih_{nm}", [128, FT * G], WDT,
                                     kind="ExternalInput")
        whh_d_d[nm] = nc.dram_tensor(f"whh_{nm}", [128, KT * G], WDT,
                                     kind="ExternalInput")
    linT_d = nc.dram_tensor("linT", [128, KT * F], WDT, kind="ExternalInput")
    dmask_d = nc.dram_tensor("dmask", [B, B * TCAT], BF, kind="ExternalInput")
    tpT_d = nc.dram_tensor("tpT", [128, KT * P], BF, kind="ExternalInput")
    bias_d = {}
    for key, flag, shp in (("b_tf", has_btf, [1, F]),
                           ("b_e", has_be, [1, G]),
                           ("b_p", has_bp, [1, G]),
                           ("b_d", has_bd, [1, G]),
                           ("b_lin", has_blin, [1, F]),
                           ("b_tp", has_btp, [1, P])):
        if flag:
            bias_d[key] = nc.dram_tensor(key, shp, BF, kind="ExternalInput")
    out_d = nc.dram_tensor("oT", [P, T_OUT * B], FP32, kind="ExternalOutput")
    dbg_d = {}
    if DEBUG_TAPS:
        MD = F8 if FP8 else BF
        for nm, shp, dt in (("dbg_cat", [128, KT * B * TCAT], BF),
                            ("dbg_ce", [128, KT * B], FP32),
                            ("dbg_scores", [B, TCAT], FP32),
                            ("dbg_attw", [B, TCAT], BF),
                            ("dbg_atth", [128, KT * B], MD),
                            ("dbg_gates", [128, MT * B], BF),
                            ("dbg_u", [128, KT * B], FP32),
                            ("dbg_h", [128, KT * B], BF),
                            ("dbg_inp", [128, FT * B], MD)):
            dbg_d[nm] = nc.dram_tensor(nm, shp, dt, kind="ExternalOutput")

    with nc.allow_low_precision("fp8/bf16 gate kernel, tolerance 2e-2"), \
         tile.TileContext(nc) as tc:
        with (
            tc.tile_pool(name="singles", bufs=1) as singles,
            tc.tile_pool(name="wih_pool", bufs=2 if FP8 else 1) as wih_pool,
            tc.tile_pool(name="whh_pool", bufs=2 if FP8 else 1) as whh_pool,
            tc.tile_pool(name="gx_pool", bufs=2) as gx_pool,
            tc.tile_pool(name="ew_pool", bufs=6 if FP8 else 4) as ew_pool,
            tc.tile_pool(name="pA_pool", bufs=1, space="PSUM") as pA_pool,
            tc.tile_pool(name="pB_pool", bufs=1, space="PSUM") as pB_pool,
            tc.tile_pool(name="pC_pool", bufs=1, space="PSUM") as pC_pool,
            tc.tile_pool(name="mps_pool", bufs=2, space="PSUM") as mps_pool,
        ):
            def body(_it=None):
                # ------------- constant/static loads --------------------
                tfT = singles.tile([P, F], BF, tag="tfT")
                nc.sync.dma_start(out=tfT, in_=tfT_d[:, :])
                xT = singles.tile([P, T_IN * B], BF, tag="xT")
                nc.sync.dma_start(out=xT, in_=xT_d[:, :])
                zT = singles.tile([P, T_IN * B], BF, tag="zT")
                nc.sync.dma_start(out=zT, in_=zT_d[:, :])

                wih_e = wih_pool.tile([128, FT * G], WDT, tag="wih")
                for kt in range(FT):
                    nc.sync.dma_start(out=wih_e[:, kt * G:(kt + 1) * G],
                                      in_=wih_d_d["e"][:, kt * G:(kt + 1) * G])
                wih_p = wih_pool.tile([128, FT * G], WDT, tag="wih")
                for kt in range(FT):
                    nc.sync.dma_start(out=wih_p[:, kt * G:(kt + 1) * G],
                                      in_=wih_d_d["p"][:, kt * G:(kt + 1) * G])
                whh_e = whh_pool.tile([128, KT * G], WDT, tag="whh")
                for kt in range(KT):
                    nc.sync.dma_start(out=whh_e[:, kt * G:(kt + 1) * G],
                                      in_=whh_d_d["e"][:, kt * G:(kt + 1) * G])
                whh_p = whh_pool.tile([128, KT * G], WDT, tag="whh")
                for kt in range(KT):
                    nc.sync.dma_start(out=whh_p[:, kt * G:(kt + 1) * G],
                                      in_=whh_d_d["p"][:, kt * G:(kt + 1) * G])

                residT = singles.tile([P, T_OUT * B], FP32, tag="residT")
                nc.sync.dma_start(out=residT, in_=residT_d[:, :])
                linT = singles.tile([128, KT * F], WDT, tag="linT")
                for kt in range(KT):
                    nc.sync.dma_start(out=linT[:, kt * F:(kt + 1) * F],
                                      in_=linT_d[:, kt * F:(kt + 1) * F])
                tpT = singles.tile([128, KT * P], BF, tag="tpT")
                nc.sync.dma_start(out=tpT, in_=tpT_d[:, :])
                dmask = singles.tile([B, B * TCAT], BF, tag="dmask")
                nc.sync.dma_start(out=dmask, in_=dmask_d[:, :])

                bias_sb = {}
                for key, dram in bias_d.items():
                    t = singles.tile(list(dram.shape), BF, tag=key)
                    nc.sync.dma_start(out=t, in_=dram[:, :])
                    bias_sb[key] = t

                ones32 = singles.tile([B, 128], BF, tag="ones32")
                nc.vector.memset(ones32, 1.0)
                if any_bias:
                    ones_n = singles.tile([1, T_IN * B], BF, tag="ones_n")
                    nc.vector.memset(ones_n, 1.0)

                cat = singles.tile([128, KT, B, TCAT], BF, tag="cat")
                c_e = singles.tile([128, KT, B], FP32, tag="c_e")
                c_p = singles.tile([128, KT, B], FP32, tag="c_p")
                u_d = singles.tile([128, KT, B], FP32, tag="u_d")  # 2*c_dec

                c_bf = singles.tile([128, KT, B], BF, tag="c_bf")
                h8e = singles.tile([128, KT, B], MDT, tag="h8e")
                h8p = singles.tile([128, KT, B], MDT, tag="h8p")
                h8d = singles.tile([128, KT, B], MDT, tag="h8d")
                atth8 = singles.tile([128, KT, B], MDT, tag="atth8")
                inp8 = singles.tile([128, FT, B], MDT, tag="inp8")
                e_bc = singles.tile([128, B * TCAT], BF, tag="e_bc")
                masked32 = singles.tile([B, B * TCAT], FP32, tag="masked32")
                scoresbt = singles.tile([B, TCAT], FP32, tag="scoresbt")
                neg_mx = singles.tile([B, 1], FP32, tag="neg_mx")
                e32 = singles.tile([B, TCAT], FP32, tag="e32")
                ssum32 = singles.tile([B, 1], FP32, tag="ssum32")
                rs32 = singles.tile([B, 1], FP32, tag="rs32")
                attw4 = singles.tile([B, TCAT], BF, tag="attw4")
                aw_m = singles.tile([B, B * TCAT], BF, tag="aw_m")
                prod2 = singles.tile([128, B * TCAT], BF, tag="prod2")
                oT_sb = singles.tile([P, T_OUT * B], FP32, tag="oT_sb")

                xf8 = singles.tile([128, FT, T_IN * B], MDT, tag="xf8")
                zf8 = singles.tile([128, FT, T_IN * B], MDT, tag="zf8")

                # ------------- ToFeature --------------------------------
                def to_feature(src, dst):
                    for ft in range(FT):
                        ps = mps_pool.tile([128, T_IN * B], FP32, tag="mps")
                        nc.tensor.matmul(ps, tfT[:, ft * 128:(ft + 1) * 128],
                                         src[:, :], start=True,
                                         stop=not has_btf)
                        if has_btf:
                            nc.tensor.matmul(
                                ps,
                                bias_sb["b_tf"][0:1, ft * 128:(ft + 1) * 128],
                                ones_n[0:1, :], start=False, stop=True)
                        nc.vector.tensor_scalar_mul(dst[:, ft, :], ps, sx)

                to_feature(xT, xf8)
                to_feature(zT, zf8)

                # ------------- encoder gates_x precompute ----------------
                # gx layout: (128, T_IN, MT, B); values carry the SG scale.
                def gates_x(wih_sb, src, dst, bkey):
                    for mt in range(MT):
                        ps = mps_pool.tile([128, T_IN * B], FP32, tag="mps")
                        for kt in range(FT):
                            nc.tensor.matmul(
                                ps,
                                wih_sb[:, kt * G + mt * 128:
                                       kt * G + (mt + 1) * 128],
                                src[:, kt, :],
                                start=(kt == 0),
                                stop=(kt == FT - 1 and bkey is None),
                            )
                        if bkey is not None:
                            nc.tensor.matmul(
                                ps, bias_sb[bkey][0:1, mt * 128:(mt + 1) * 128],
                                ones_n[0:1, :], start=False, stop=True)
                        nc.vector.tensor_copy(
                            out=dst[:, :, mt, :],
                            in_=ps.rearrange("p (t b) -> p t b", b=B))

                gx_e = gx_pool.tile([128, T_IN, MT, B], BF, tag="gx")
                gates_x(wih_e, xf8, gx_e, "b_e" if has_be else None)
                gx_p = gx_pool.tile([128, T_IN, MT, B], BF, tag="gx")
                gates_x(wih_p, zf8, gx_p, "b_p" if has_bp else None)

                def pe_filler(dep_ap):
                    # Tiny matmul dependent on a just-produced DVE/ACT tile;
                    # keeps the PE HAM window busy during serial DVE/ACT
                    # chains so real matmuls resume at 2.4 GHz.
                    if not warm_fillers:
                        return
                    fps = mps_pool.tile([1, 8], FP32, tag="mps")
                    nc.tensor.matmul(fps[:, 0:1], dep_ap, dep_ap,
                                     start=True, stop=True)

                # ------------- interleaved encoders ----------------------
                # Per-chain scratch (ew_pool cycles 4 bufs; keep chains on
                # distinct tags to be safe).
                def enc_step(t, whh_sb, gx, c_tile, h8, slot, ppool):
                    if t == 0:
                        sio = ew_pool.tile([128, 3 * KT, B], FP32, tag="ew")
                        tg = ew_pool.tile([128, KT, B], FP32, tag="ew")
                        nc.scalar.activation(out=sio, in_=gx[:, 0, 0:3 * KT, :],
                                             func=ACT.Sigmoid, scale=sg_inv)
                        nc.scalar.activation(out=tg, in_=gx[:, 0, 3 * KT:, :],
                                             func=ACT.Tanh, scale=sg_inv)
                        # c = sig(i)*tanh(g);  h = sig(o)*tanh(c)
                        nc.vector.tensor_mul(c_tile, sio[:, 0:KT, :], tg)
                        th = ew_pool.tile([128, KT, B], FP32, tag="ew")
                        nc.scalar.activation(out=th, in_=c_tile, func=ACT.Tanh)
                        h = cat[:, :, :, slot]
                        nc.vector.tensor_mul(h, sio[:, 2 * KT:3 * KT, :], th)
                        nc.vector.tensor_scalar_mul(h8, h, sh)
                        return
                    gps = ppool.tile([128, MT, B], FP32, tag="gp")
                    for mt in range(MT):
                        for kt in range(KT):
                            nc.tensor.matmul(
                                gps[:, mt, :],
                                whh_sb[:, kt * G + mt * 128:
                                       kt * G + (mt + 1) * 128],
                                h8[:, kt, :],
                                start=(kt == 0), stop=(kt == KT - 1),
                            )
                    gsum = ew_pool.tile([128, MT, B], BF, tag="gsum")
                    nc.vector.tensor_add(gsum, gps, gx[:, t, :, :])
                    sio = ew_pool.tile([128, 3 * KT, B], FP32, tag="ew")
                    tg = ew_pool.tile([128, KT, B], FP32, tag="ew")
                    nc.scalar.activation(out=sio, in_=gsum[:, 0:3 * KT, :],
                                         func=ACT.Sigmoid, scale=sg_inv)
                    nc.scalar.activation(out=tg, in_=gsum[:, 3 * KT:, :],
                                         func=ACT.Tanh, scale=sg_inv)
                    nc.vector.tensor_mul(tg, sio[:, 0:KT, :], tg)     # i*g
                    nc.vector.tensor_mul(c_tile, sio[:, KT:2 * KT, :], c_tile)
                    nc.vector.tensor_add(c_tile, c_tile, tg)
                    th = ew_pool.tile([128, KT, B], FP32, tag="ew")
                    nc.scalar.activation(out=th, in_=c_tile, func=ACT.Tanh)
                    h = cat[:, :, :, slot]
                    nc.vector.tensor_mul(h, sio[:, 2 * KT:3 * KT, :], th)
                    nc.vector.tensor_scalar_mul(h8, h, sh)

                if "enc" not in ablate:
                    if FP8:
                        # interleaved: each chain's elementwise tail hides
                        # under the other chain's whh matmuls
                        for t in range(T_IN):
                            enc_step(t, whh_e, gx_e, c_e, h8e, t, pB_pool)
                            enc_step(t, whh_p, gx_p, c_p, h8p, T_IN + 1 + t,
                                     pC_pool)
                    else:
                        # bf16: whh pool is single-buffered, chains must be
                        # sequential
                        for t in range(T_IN):
                            enc_step(t, whh_e, gx_e, c_e, h8e, t, pB_pool)
                        for t in range(T_IN):
                            enc_step(t, whh_p, gx_p, c_p, h8p, T_IN + 1 + t,
                                     pC_pool)
                else:
                    nc.vector.memset(cat, 0.01)
                    nc.vector.memset(c_e, 0.01)
                    nc.vector.memset(c_p, 0.01)

                # decoder weights (reuse the e-slots of the pools)
                wih_dd = wih_pool.tile([128, FT * G], WDT, tag="wih")
                for kt in range(FT):
                    nc.sync.dma_start(out=wih_dd[:, kt * G:(kt + 1) * G],
                                      in_=wih_d_d["d"][:, kt * G:(kt + 1) * G])
                whh_dd = whh_pool.tile([128, KT * G], WDT, tag="whh")
                for kt in range(KT):
                    nc.sync.dma_start(out=whh_dd[:, kt * G:(kt + 1) * G],
                                      in_=whh_d_d["d"][:, kt * G:(kt + 1) * G])

                dec_hs = gx_pool.tile([128, KT, T_OUT, B], BF, tag="gx")

                def tap(name, src):
                    if not DEBUG_TAPS or name not in dbg_d:
                        return
                    if len(src.shape) > 2:
                        dims = [chr(97 + i) for i in range(len(src.shape) - 1)]
                        src = src.rearrange(
                            "p " + " ".join(dims)
                            + " -> p (" + " ".join(dims) + ")")
                    if src.space != bass.MemorySpace.SBUF:
                        st = singles.tile([src.shape[0], src.shape[1]],
                                          dbg_d[name].dtype,
                                          tag=name + "_st")
                        nc.vector.tensor_copy(out=st, in_=src)
                        src = st
                    nc.sync.dma_start(out=dbg_d[name][:, :], in_=src)

                # ------------- decoder ----------------------------------
                # state init: h = enc h_n (slot 10), u = 2*c_e, c_bf = c_e
                nc.vector.tensor_copy(out=cat[:, :, :, SLOT_DEC],
                                      in_=cat[:, :, :, T_IN - 1])
                tap("dbg_cat", cat)
                tap("dbg_ce", c_e)
                nc.vector.tensor_scalar_mul(h8d, cat[:, :, :, SLOT_DEC], sh)
                nc.vector.tensor_scalar_mul(u_d, c_e, 2.0)
                nc.vector.tensor_copy(out=c_bf, in_=c_e)

                nhalf = B // 2

                dec_steps = 0 if "dec" in ablate else T_OUT
                for t in range(dec_steps):
                    # --- PE: scores (needs c_bf from prev tail) ----------
                    scd = pA_pool.tile([B, 2, 512], FP32, tag="pA")
                    if "attn" not in ablate:
                        for kt in range(KT):
                            nc.tensor.matmul(
                                scd[:, 0, 0:nhalf * TCAT],
                                c_bf[:, kt, :], cat[:, kt, 0:nhalf, :],
                                start=(kt == 0), stop=(kt == KT - 1))
                            nc.tensor.matmul(
                                scd[:, 1, 0:nhalf * TCAT],
                                c_bf[:, kt, :], cat[:, kt, nhalf:B, :],
                                start=(kt == 0), stop=(kt == KT - 1))

                    # --- PE: lin (needs h8 from prev tail) ---------------
                    ips = mps_pool.tile([128, FT, B], FP32, tag="mps")
                    for mt in range(FT):
                        for kt in range(KT):
                            nc.tensor.matmul(
                                ips[:, mt, :],
                                linT[:, kt * F + mt * 128:
                                     kt * F + (mt + 1) * 128],
                                h8d[:, kt, :],
                                start=(kt == 0),
                                stop=(kt == KT - 1 and not has_blin),
                            )
                        if has_blin:
                            nc.tensor.matmul(
                                ips[:, mt, :],
                                bias_sb["b_lin"][0:1, mt * 128:(mt + 1) * 128],
                                ones_n[0:1, 0:B], start=False, stop=True)
                    # inp8 = ips * SX/SG   (true inp * SX)
                    nc.vector.tensor_scalar_mul(inp8, ips, sx * sg_inv)
                    if t == 0:
                        tap("dbg_inp", inp8)

                    # --- PE: wih half of gates (group left open) ---------
                    # start=True clears has_written for the WHOLE psum bank,
                    # so only the very first matmul per bank may carry it;
                    # every other slice's first write lands on cleared bits
                    # and overwrites (per-element init), later ones
                    # accumulate. One stop per bank closes the group.
                    MT_BANK = 2048 // (B * 4)   # m-tiles per psum bank
                    gps = pB_pool.tile([128, MT, B], FP32, tag="gp")
                    for mt in range(MT):
                        for kt in range(FT):
                            nc.tensor.matmul(
                                gps[:, mt, :],
                                wih_dd[:, kt * G + mt * 128:
                                       kt * G + (mt + 1) * 128],
                                inp8[:, kt, :],
                                start=(kt == 0 and mt % MT_BANK == 0),
                                stop=False, skip_group_check=True)
                        if has_bd:
                            nc.tensor.matmul(
                                gps[:, mt, :],
                                bias_sb["b_d"][0:1, mt * 128:(mt + 1) * 128],
                                ones_n[0:1, 0:B], start=False, stop=False,
                                skip_group_check=True)

                    if "attn" not in ablate:
                        # --- DVE/ACT: softmax over 21 slots --------------
                        nc.vector.tensor_mul(
                            masked32.rearrange("p (c n) -> p c n", c=2),
                            scd[:, :, 0:nhalf * TCAT],
                            dmask.rearrange("p (c n) -> p c n", c=2))
                        nc.vector.tensor_reduce(
                            scoresbt,
                            masked32.rearrange("p (b t) -> p t b", t=TCAT),
                            axis=AXX.X, op=ALU.add)
                        pe_filler(scoresbt[:, 0:1])
                        if t == 0:
                            tap("dbg_scores", scoresbt)
                        nc.vector.tensor_reduce(
                            neg_mx, scoresbt, axis=AXX.X,
                            op=ALU.max, negate=True)
                        nc.scalar.activation(
                            out=e32, in_=scoresbt, func=ACT.Exp,
                            bias=neg_mx, accum_out=ssum32)
                        pe_filler(e32[:, 0:1])
                        nc.vector.reciprocal(rs32, ssum32)
                        # attw4 = softmax * SH  (folds the fp8 atth scale)
                        nc.vector.tensor_scalar(
                            attw4, e32, rs32, sh, ALU.mult, ALU.mult)
                        nc.vector.tensor_mul(
                            aw_m, dmask,
                            attw4.unsqueeze(1).to_broadcast((B, B, TCAT)))
                        # broadcast attw to all partitions: ones-matmul
                        eall = pA_pool.tile([128, 2, 512], FP32, tag="pA")
                        nc.tensor.matmul(eall[:, 0, 0:nhalf * TCAT],
                                         ones32, aw_m[:, 0:nhalf * TCAT],
                                         start=True, stop=True)
                        nc.tensor.matmul(eall[:, 1, 0:nhalf * TCAT],
                                         ones32, aw_m[:, nhalf * TCAT:],
                                         start=True, stop=True)
                        nc.scalar.copy(
                            out=e_bc.rearrange("p (c n) -> p c n", c=2),
                            in_=eall[:, :, 0:nhalf * TCAT])

                        # context, k-tile pipelined against whh matmuls:
                        # atth8[kt] = sum_t cat[:,kt]*attw4  (fp8, *SH).
                        # The reduce writes bf16 (2-byte keeps the DVE 2x
                        # mode); fp8 needs a separate tiny cast.
                        for kt in range(KT):
                            nc.vector.tensor_mul(
                                prod2, cat[:, kt, :, :],
                                e_bc.rearrange("p (b t) -> p b t", t=TCAT))
                            if FP8:
                                ctxb = ew_pool.tile([128, B], BF, tag="ctxb")
                                nc.vector.tensor_reduce(
                                    ctxb,
                                    prod2.rearrange("p (b t) -> p b t",
                                                    t=TCAT),
                                    axis=AXX.X, op=ALU.add)
                                nc.vector.tensor_copy(out=atth8[:, kt, :],
                                                      in_=ctxb)
                            else:
                                nc.vector.tensor_reduce(
                                    atth8[:, kt, :],
                                    prod2.rearrange("p (b t) -> p b t",
                                                    t=TCAT),
                                    axis=AXX.X, op=ALU.add)
                            for mt in range(MT):
                                nc.tensor.matmul(
                                    gps[:, mt, :],
                                    whh_dd[:, kt * G + mt * 128:
                                           kt * G + (mt + 1) * 128],
                                    atth8[:, kt, :],
                                    start=False,
                                    stop=(kt == KT - 1
                                          and mt % MT_BANK == MT_BANK - 1),
                                    skip_group_check=True)
                    else:
                        if t == 0:
                            nc.vector.memset(atth8, 0.01)
                        for kt in range(KT):
                            for mt in range(MT):
                                nc.tensor.matmul(
                                    gps[:, mt, :],
                                    whh_dd[:, kt * G + mt * 128:
                                           kt * G + (mt + 1) * 128],
                                    atth8[:, kt, :],
                                    start=False,
                                    stop=(kt == KT - 1
                                          and mt % MT_BANK == MT_BANK - 1),
                                    skip_group_check=True)

                    if t == 0:
                        tap("dbg_attw", attw4)
                        tap("dbg_atth", atth8)
                        tap("dbg_gates", gps)

                    # --- tail: tanh-only gates ---------------------------
                    # sio = tanh(gates_ifo/2), tg = tanh(g)
                    sio = ew_pool.tile([128, 3 * KT, B], FP32, tag="ew")
                    tg = ew_pool.tile([128, KT, B], FP32, tag="ew")
                    nc.scalar.activation(out=sio, in_=gps[:, 0:3 * KT, :],
                                         func=ACT.Tanh, scale=0.5 * sg_inv)
                    nc.scalar.activation(out=tg, in_=gps[:, 3 * KT:, :],
                                         func=ACT.Tanh, scale=sg_inv)
                    pe_filler(sio[:, 0:1, 0:1])
                    # u' = (tf+1)*u/2 + (ti+1)*g'   (u = 2c)
                    A = ew_pool.tile([128, KT, B], FP32, tag="ew")
                    Bt = ew_pool.tile([128, KT, B], FP32, tag="ew")
                    nc.vector.scalar_tensor_tensor(
                        out=A, in0=sio[:, 0:KT, :], scalar=1.0, in1=tg,
                        op0=ALU.add, op1=ALU.mult)
                    nc.vector.scalar_tensor_tensor(
                        out=Bt, in0=sio[:, KT:2 * KT, :], scalar=1.0, in1=u_d,
                        op0=ALU.add, op1=ALU.mult)
                    nc.vector.scalar_tensor_tensor(
                        out=u_d, in0=Bt, scalar=0.5, in1=A,
                        op0=ALU.mult, op1=ALU.add)
                    # c_bf for next scores; th = tanh(c) = tanh(u/2)
                    nc.scalar.mul(c_bf, u_d, 0.5)
                    th = ew_pool.tile([128, KT, B], FP32, tag="ew")
                    nc.scalar.activation(out=th, in_=u_d, func=ACT.Tanh,
                                         scale=0.5)
                    # v = (to+1)*th = 2h
                    v = ew_pool.tile([128, KT, B], FP32, tag="ew")
                    nc.vector.scalar_tensor_tensor(
                        out=v, in0=sio[:, 2 * KT:3 * KT, :], scalar=1.0,
                        in1=th, op0=ALU.add, op1=ALU.mult)
                    nc.scalar.mul(cat[:, :, :, SLOT_DEC], v, 0.5)
                    nc.vector.tensor_scalar_mul(h8d, v, 0.5 * sh)
                    nc.vector.tensor_scalar_mul(dec_hs[:, :, t, :], v, 0.5)
                    if t == 0:
                        tap("dbg_u", u_d)
                        tap("dbg_h", cat[:, :, :, SLOT_DEC])

                # ------------- ToPose + residual ------------------------
                if "dec" in ablate:
                    return
                ops = pA_pool.tile([P, 2, 512], FP32, tag="pA")
                chunks = [(0, 13), (13, 12)]
                for ci, (t0, tn) in enumerate(chunks):
                    n = tn * B
                    for kt in range(KT):
                        nc.tensor.matmul(
                            ops[:, ci, 0:n],
                            tpT[:, kt * P:(kt + 1) * P],
                            dec_hs[:, kt, t0:t0 + tn, :].rearrange(
                                "p t b -> p (t b)"),
                            start=(kt == 0),
                            stop=(kt == KT - 1 and not has_btp))
                    if has_btp:
                        nc.tensor.matmul(
                            ops[:, ci, 0:n], bias_sb["b_tp"][0:1, :],
                            ones_n[0:1, 0:n], start=False, stop=True)
                    nc.vector.tensor_add(
                        oT_sb[:, t0 * B:t0 * B + n],
                        ops[:, ci, 0:n],
                        residT[:, t0 * B:t0 * B + n])
                nc.sync.dma_start(out=out_d[:, :], in_=oT_sb)

            if loop_iters > 1:
                with tc.For_i(0, loop_iters, 1, name="rep"):
                    body()
            else:
                body()

    return nc


# ------------------------------------------------------------- entry point

_model_cache = {}


def _get_model(key):
    if key not in _model_cache:
        _model_cache[key] = build_model(key)
    return _model_cache[key]


def make_in_maps(inputs):
    """Host-side packing: returns per-core input maps."""
    w = _prep_weights(inputs)
    flags = _bias_flags(w)
    x = np.asarray(inputs["x"], dtype=np.float32)
    z = np.asarray(inputs["z"], dtype=np.float32)
    fr = np.asarray(inputs["for_resid"], dtype=np.float32)

    dmask = np.zeros((B, B, TCAT), dtype=np.float32)
    for b in range(B):
        dmask[b, b, :] = 1.0
    shared = {
        "tfT": w["tfT"], "linT": w["linT"], "tpT": w["tpT"],
        "dmask": np.ascontiguousarray(
            dmask.reshape(B, B * TCAT)).astype(BF16),
    }
    for nm in ("e", "p", "d"):
        shared[f"wih_{nm}"] = w[f"wih_{nm}"]
        shared[f"whh_{nm}"] = w[f"whh_{nm}"]
    names = ("b_tf", "b_e", "b_p", "b_d", "b_lin", "b_tp")
    for f, name in zip(flags, names):
        if f:
            shared[name] = np.ascontiguousarray(
                w[name][None, :]).astype(BF16)

    in_maps = []
    for c in range(N_CORES):
        sl = slice(c * B, (c + 1) * B)
        m = dict(shared)
        m["xT"] = np.ascontiguousarray(
            x[sl].transpose(2, 1, 0).reshape(P, T_IN * B)).astype(BF16)
        m["zT"] = np.ascontiguousarray(
            z[sl].transpose(2, 1, 0).reshape(P, T_IN * B)).astype(BF16)
        m["residT"] = np.ascontiguousarray(
            fr[sl].transpose(2, 1, 0).reshape(P, T_OUT * B))
        in_maps.append(m)
    return in_maps, flags


def unshard_output(results):
    outs = []
    for c in range(N_CORES):
        oT = np.asarray(results[c]["oT"])  # (66, 800)
        outs.append(oT.reshape(P, T_OUT, B).transpose(2, 1, 0))
    return np.ascontiguousarray(np.concatenate(outs, axis=0),
                                dtype=np.float32)


def kernel(**inputs) -> np.ndarray:
    in_maps, flags = make_in_maps(inputs)
    nc = _get_model(flags)
    res = run_bass_kernel_spmd(nc, in_maps, core_ids=list(range(N_CORES)))
    return unshard_output(res.results)
